# revision 15
# baseline (speedup 1.0000x reference)
"""Expert-parallel MoE layer for 8 Trainium2 NeuronCores (Bass/Tile).

Strategy
--------
The reference computes a router (logits + fixed gumbel noise, softmax,
top-k) and then a dense all-expert SwiGLU, finally combining only the
K=4 selected experts per token.  Only the selected (token, expert)
pairs contribute to the output, so we:

1. Reproduce the router on host with the exact same jax ops as the
   reference (same process / same default backend -> bit-identical
   gates and indices).
2. Bin tokens by expert, pad each expert batch to a multiple of 256,
   and pack the 16 experts into 16 uniform slots = 2 phases x 8 cores
   (phase sizes chosen per call from the actual routing, biggest 8
   experts in phase 0).  Every core runs the identical SPMD program:
   two expert jobs, phase sizes fixed at build time.
3. On each core: H1T = W1^T-chunks @ XT, H2T likewise, AT =
   silu(H1T+b1)*(H2T+b2), YT = Wout^T-chunks @ AT + bout -- all in
   transposed [feature, token] layout so weights stay in natural
   [D, D] layout as the stationary matmul operand.  Matmuls run as
   float32r (full PE rate at moving-dim >= 256, ~FP22 precision).
4. Host scatters YT back: out[tok_e] += YT[:, :n_e].T * gate_e.
"""

import os

import numpy as np

N_TOK, D, E, K = 8192, 1024, 16, 4
N_CORES = 8
QUANT = 256  # expert batch padding quantum
BLOCK = 512  # moving-operand (token) block width
N_DCHUNK = D // 128  # 8 contraction chunks of 128

# Diagnostics for the dev harness (test.py); the grading harness only
# calls kernel().
LAST = {}

_PROG_CACHE = {}


# --------------------------------------------------------------------------
# Router: must match reference.py bit-for-bit.  Same jax calls, same
# default device as the reference run in this process.
# --------------------------------------------------------------------------
def _route(x, w_router):
    import jax
    import jax.numpy as jnp

    logits = jnp.dot(jnp.asarray(x).astype(jnp.float32), jnp.asarray(w_router))
    noise = jax.random.gumbel(jax.random.PRNGKey(0), logits.shape) * 0.05
    probs = jax.nn.softmax(logits + noise)
    gates, indices = jax.lax.top_k(probs, K)
    return np.asarray(gates), np.asarray(indices)


def _pad(c):
    return max(QUANT, ((c + QUANT - 1) // QUANT) * QUANT)


def _plan(counts):
    """16 slots = 2 phases x 8 cores, uniform width per phase."""
    padded = [_pad(c) for c in counts]
    order = sorted(range(E), key=lambda e: -padded[e])
    groups = [order[:N_CORES], order[N_CORES:]]
    sizes = [max(padded[e] for e in g) for g in groups]
    return groups, sizes


def _blocks(S):
    out = []
    off = 0
    while off < S:
        n = min(BLOCK, S - off)
        out.append((off, n))
        off += n
    return out


# --------------------------------------------------------------------------
# Bass program: identical on all 8 cores; per-core data differs.
# --------------------------------------------------------------------------
def _build(job_sizes):
    import concourse.bacc as bacc
    import concourse.mybir as mybir
    from concourse import tile

    f32 = mybir.dt.float32
    f32r = mybir.dt.float32r
    AF = mybir.ActivationFunctionType

    nc = bacc.Bacc(None, target_bir_lowering=False, debug=False)

    params = []
    for j, S in enumerate(job_sizes):
        params.append(
            dict(
                xt=nc.declare_dram_parameter(f"xt{j}", [D, S], f32, isOutput=False),
                w1=nc.declare_dram_parameter(f"w1_{j}", [D, D], f32, isOutput=False),
                w2=nc.declare_dram_parameter(f"w2_{j}", [D, D], f32, isOutput=False),
                wo=nc.declare_dram_parameter(f"wo_{j}", [D, D], f32, isOutput=False),
                bias=nc.declare_dram_parameter(f"bias{j}", [128, 24], f32, isOutput=False),
                yt=nc.declare_dram_parameter(f"yt{j}", [D, S], f32, isOutput=True),
            )
        )

    with tile.TileContext(nc) as tc:
        with (
            tc.tile_pool(name="wpool", bufs=1) as wpool,
            tc.tile_pool(name="w1pool", bufs=2) as w1pool,
            tc.tile_pool(name="bpool", bufs=2) as bpool,
            tc.tile_pool(name="xpool", bufs=2) as xpool,
            tc.tile_pool(name="atpool", bufs=2) as atpool,
            tc.tile_pool(name="tpool", bufs=2) as tpool,
            tc.tile_pool(name="opool", bufs=3) as opool,
            tc.tile_pool(name="psA", bufs=3, space="PSUM") as psA,
            tc.tile_pool(name="psB", bufs=3, space="PSUM") as psB,
            tc.tile_pool(name="psC", bufs=2, space="PSUM") as psC,
        ):
            def load_x(p, off, NB):
                xsb = xpool.tile([128, N_DCHUNK * NB], f32r, tag="xt")
                for d in range(N_DCHUNK):
                    nc.sync.dma_start(
                        xsb[:, d * NB : (d + 1) * NB],
                        p["xt"][d * 128 : (d + 1) * 128, off : off + NB].bitcast(f32r),
                    )
                return xsb

            for j, S in enumerate(job_sizes):
                p = params[j]
                blocks = _blocks(S)

                bsb = bpool.tile([128, 24], f32, tag="bias")
                nc.sync.dma_start(bsb[:], p["bias"][:])
                # First token block before the weights so PE can start as
                # soon as the first weight columns land.
                xsb = load_x(p, *blocks[0])

                # Weights as per-output-column tiles: one DMA delivers all 8
                # contraction chunks for a single 128-wide output column, so
                # the first matmul group only waits for ~1 MB, and the next
                # job's weight loads stream in behind the current job's tail.
                w1v = p["w1"][:].rearrange("(d p) h -> p d h", p=128).bitcast(f32r)
                w2v = p["w2"][:].rearrange("(d p) h -> p d h", p=128).bitcast(f32r)
                wov = p["wo"][:].rearrange("(d p) h -> p d h", p=128).bitcast(f32r)
                w1h, w2h, woh = [], [], []
                for h in range(N_DCHUNK):
                    t1w = w1pool.tile([128, N_DCHUNK * 128], f32r, tag=f"w1h{h}")
                    t2w = wpool.tile([128, N_DCHUNK * 128], f32r, tag=f"w2h{h}")
                    nc.sync.dma_start(
                        t1w[:].rearrange("p (d c) -> p d c", d=N_DCHUNK),
                        w1v[:, :, h * 128 : (h + 1) * 128],
                    )
                    nc.sync.dma_start(
                        t2w[:].rearrange("p (d c) -> p d c", d=N_DCHUNK),
                        w2v[:, :, h * 128 : (h + 1) * 128],
                    )
                    w1h.append(t1w)
                    w2h.append(t2w)
                for do in range(N_DCHUNK):
                    tw = wpool.tile([128, N_DCHUNK * 128], f32r, tag=f"woh{do}")
                    nc.sync.dma_start(
                        tw[:].rearrange("p (d c) -> p d c", d=N_DCHUNK),
                        wov[:, :, do * 128 : (do + 1) * 128],
                    )
                    woh.append(tw)

                for bi, (off, NB) in enumerate(blocks):
                    if bi > 0:
                        xsb = load_x(p, off, NB)
                    atsb = atpool.tile([128, N_DCHUNK * NB], f32r, tag="at")
                    for h in range(N_DCHUNK):
                        pa = psA.tile([128, NB], f32, tag="pa")
                        pb = psB.tile([128, NB], f32, tag="pb")
                        for d in range(N_DCHUNK):
                            nc.tensor.matmul(
                                pa[:],
                                w1h[h][:, d * 128 : (d + 1) * 128],
                                xsb[:, d * NB : (d + 1) * NB],
                                start=(d == 0),
                                stop=(d == N_DCHUNK - 1),
                            )
                        for d in range(N_DCHUNK):
                            nc.tensor.matmul(
                                pb[:],
                                w2h[h][:, d * 128 : (d + 1) * 128],
                                xsb[:, d * NB : (d + 1) * NB],
                                start=(d == 0),
                                stop=(d == N_DCHUNK - 1),
                            )
                        t1 = tpool.tile([128, NB], f32, tag="t1")
                        t2 = tpool.tile([128, NB], f32, tag="t2")
                        nc.scalar.activation(t1[:], pa[:], AF.Silu, bias=bsb[:, h : h + 1])
                        nc.scalar.activation(t2[:], pb[:], AF.Identity, bias=bsb[:, 8 + h : 9 + h])
                        nc.vector.tensor_mul(atsb[:, h * NB : (h + 1) * NB], t1[:], t2[:])
                    for do in range(N_DCHUNK):
                        pc = psC.tile([128, NB], f32, tag="pc")
                        for h in range(N_DCHUNK):
                            nc.tensor.matmul(
                                pc[:],
                                woh[do][:, h * 128 : (h + 1) * 128],
                                atsb[:, h * NB : (h + 1) * NB],
                                start=(h == 0),
                                stop=(h == N_DCHUNK - 1),
                            )
                        ot = opool.tile([128, NB], f32, tag="ot")
                        nc.scalar.activation(ot[:], pc[:], AF.Identity, bias=bsb[:, 16 + do : 17 + do])
                        nc.sync.dma_start(
                            p["yt"][do * 128 : (do + 1) * 128, off : off + NB], ot[:]
                        )
    nc.compile()
    return nc


def _get_prog(job_sizes):
    key = tuple(job_sizes)
    if key not in _PROG_CACHE:
        _PROG_CACHE[key] = _build(job_sizes)
    return _PROG_CACHE[key]


# --------------------------------------------------------------------------
# Entry point
# --------------------------------------------------------------------------
def kernel(x, w_router, W1, b1, W2, b2, Wout, bout):
    from concourse.bass_utils import run_bass_kernel_spmd

    x = np.asarray(x, np.float32)
    W1 = np.asarray(W1, np.float32)
    b1 = np.asarray(b1, np.float32)
    W2 = np.asarray(W2, np.float32)
    b2 = np.asarray(b2, np.float32)
    Wout = np.asarray(Wout, np.float32)
    bout = np.asarray(bout, np.float32)

    gates, indices = _route(x, w_router)

    # Token lists per expert (ascending token order) and their gate values.
    onehot = indices == np.arange(E)[:, None, None]  # [E, N, K]
    sel = onehot.any(axis=2)  # [E, N]
    gate_of = (gates[None] * onehot).sum(axis=2)  # [E, N]
    toks = [np.nonzero(sel[e])[0] for e in range(E)]
    counts = [len(t) for t in toks]

    groups, sizes = _plan(counts)
    nc = _get_prog(sizes)

    # Per-core inputs.
    in_maps = []
    for c in range(N_CORES):
        m = {}
        for j, S in enumerate(sizes):
            e = groups[j][c]
            t = toks[e]
            xt = np.zeros((D, S), np.float32)
            if len(t):
                xt[:, : len(t)] = x[t].T
            bias = np.zeros((128, 24), np.float32)
            bias[:, 0:8] = b1[e].reshape(8, 128).T
            bias[:, 8:16] = b2[e].reshape(8, 128).T
            bias[:, 16:24] = bout[e].reshape(8, 128).T
            m[f"xt{j}"] = xt
            m[f"w1_{j}"] = np.ascontiguousarray(W1[e])
            m[f"w2_{j}"] = np.ascontiguousarray(W2[e])
            m[f"wo_{j}"] = np.ascontiguousarray(Wout[e])
            m[f"bias{j}"] = bias
        in_maps.append(m)

    trace = bool(os.environ.get("BASS_MOE_TRACE"))
    kw = {}
    if trace:
        kw = dict(trace=True, tmpdir=os.environ.get("BASS_MOE_TRACE_DIR") or None)
    res = run_bass_kernel_spmd(nc, in_maps, list(range(N_CORES)), **kw)
    LAST["result"] = res
    LAST["sizes"] = sizes

    out = np.zeros((N_TOK, D), np.float32)
    for c in range(N_CORES):
        for j in range(len(sizes)):
            e = groups[j][c]
            t = toks[e]
            if not len(t):
                continue
            yt = res.results[c][f"yt{j}"]
            out[t] += yt[:, : len(t)].T * gate_of[e, t][:, None]
    return out


# revision 16
# speedup vs baseline: 1.0156x; 1.0156x over previous
"""Expert-parallel MoE layer for 8 Trainium2 NeuronCores (Bass/Tile).

Strategy
--------
The reference computes a router (logits + fixed gumbel noise, softmax,
top-k) and then a dense all-expert SwiGLU, finally combining only the
K=4 selected experts per token.  Only the selected (token, expert)
pairs contribute to the output, so we:

1. Reproduce the router on host with the exact same jax ops as the
   reference (same process / same default backend -> bit-identical
   gates and indices).
2. Bin tokens by expert, pad each expert batch to a multiple of 256,
   and pack the 16 experts into 16 uniform slots = 2 phases x 8 cores
   (phase sizes chosen per call from the actual routing, biggest 8
   experts in phase 0).  Every core runs the identical SPMD program:
   two expert jobs, phase sizes fixed at build time.
3. On each core: H1T = W1^T-chunks @ XT, H2T likewise, AT =
   silu(H1T+b1)*(H2T+b2), YT = Wout^T-chunks @ AT + bout -- all in
   transposed [feature, token] layout so weights stay in natural
   [D, D] layout as the stationary matmul operand.  Matmuls run as
   float32r (full PE rate at moving-dim >= 256, ~FP22 precision).
4. Host scatters YT back: out[tok_e] += YT[:, :n_e].T * gate_e.
"""

import os

import numpy as np

N_TOK, D, E, K = 8192, 1024, 16, 4
N_CORES = 8
QUANT = 256  # expert batch padding quantum
BLOCK = 512  # moving-operand (token) block width
N_DCHUNK = D // 128  # 8 contraction chunks of 128

# Diagnostics for the dev harness (test.py); the grading harness only
# calls kernel().
LAST = {}

_PROG_CACHE = {}


# --------------------------------------------------------------------------
# Router: must match reference.py bit-for-bit.  Same jax calls, same
# default device as the reference run in this process.
# --------------------------------------------------------------------------
def _route(x, w_router):
    import jax
    import jax.numpy as jnp

    logits = jnp.dot(jnp.asarray(x).astype(jnp.float32), jnp.asarray(w_router))
    noise = jax.random.gumbel(jax.random.PRNGKey(0), logits.shape) * 0.05
    probs = jax.nn.softmax(logits + noise)
    gates, indices = jax.lax.top_k(probs, K)
    return np.asarray(gates), np.asarray(indices)


def _pad(c):
    return max(QUANT, ((c + QUANT - 1) // QUANT) * QUANT)


def _plan(counts):
    """16 slots = 2 phases x 8 cores, uniform width per phase."""
    padded = [_pad(c) for c in counts]
    order = sorted(range(E), key=lambda e: -padded[e])
    groups = [order[:N_CORES], order[N_CORES:]]
    sizes = [max(padded[e] for e in g) for g in groups]
    return groups, sizes


def _blocks(S):
    out = []
    off = 0
    while off < S:
        n = min(BLOCK, S - off)
        out.append((off, n))
        off += n
    return out


# --------------------------------------------------------------------------
# Bass program: identical on all 8 cores; per-core data differs.
# --------------------------------------------------------------------------
def _build(job_sizes):
    import concourse.bacc as bacc
    import concourse.mybir as mybir
    from concourse import tile

    f32 = mybir.dt.float32
    f32r = mybir.dt.float32r
    AF = mybir.ActivationFunctionType

    nc = bacc.Bacc(None, target_bir_lowering=False, debug=False)

    params = []
    for j, S in enumerate(job_sizes):
        params.append(
            dict(
                xt=nc.declare_dram_parameter(f"xt{j}", [D, S], f32, isOutput=False),
                w1=nc.declare_dram_parameter(f"w1_{j}", [D, D], f32, isOutput=False),
                w2=nc.declare_dram_parameter(f"w2_{j}", [D, D], f32, isOutput=False),
                wo=nc.declare_dram_parameter(f"wo_{j}", [D, D], f32, isOutput=False),
                bias=nc.declare_dram_parameter(f"bias{j}", [128, 24], f32, isOutput=False),
                yt=nc.declare_dram_parameter(f"yt{j}", [D, S], f32, isOutput=True),
            )
        )

    with tile.TileContext(nc) as tc:
        with (
            tc.tile_pool(name="wpool", bufs=1) as wpool,
            tc.tile_pool(name="w1pool", bufs=2) as w1pool,
            tc.tile_pool(name="bpool", bufs=2) as bpool,
            tc.tile_pool(name="xpool", bufs=2) as xpool,
            tc.tile_pool(name="atpool", bufs=2) as atpool,
            tc.tile_pool(name="tpool", bufs=2) as tpool,
            tc.tile_pool(name="opool", bufs=3) as opool,
            tc.tile_pool(name="psA", bufs=3, space="PSUM") as psA,
            tc.tile_pool(name="psB", bufs=3, space="PSUM") as psB,
            tc.tile_pool(name="psC", bufs=2, space="PSUM") as psC,
        ):
            def load_x(p, off, NB):
                xsb = xpool.tile([128, N_DCHUNK * NB], f32r, tag="xt")
                for d in range(N_DCHUNK):
                    nc.sync.dma_start(
                        xsb[:, d * NB : (d + 1) * NB],
                        p["xt"][d * 128 : (d + 1) * 128, off : off + NB].bitcast(f32r),
                    )
                return xsb

            for j, S in enumerate(job_sizes):
                p = params[j]
                blocks = _blocks(S)

                bsb = bpool.tile([128, 24], f32, tag="bias")
                nc.sync.dma_start(bsb[:], p["bias"][:])
                # First token block before the weights so PE can start as
                # soon as the first weight columns land.
                xsb = load_x(p, *blocks[0])

                # Weights as per-output-column tiles: one DMA delivers all 8
                # contraction chunks for a single 128-wide output column, so
                # the first matmul group only waits for ~1 MB, and the next
                # job's weight loads stream in behind the current job's tail.
                w1v = p["w1"][:].rearrange("(d p) h -> p d h", p=128).bitcast(f32r)
                w2v = p["w2"][:].rearrange("(d p) h -> p d h", p=128).bitcast(f32r)
                wov = p["wo"][:].rearrange("(d p) h -> p d h", p=128).bitcast(f32r)
                w1h, w2h, woh = [], [], []
                for h in range(N_DCHUNK):
                    t1w = w1pool.tile([128, N_DCHUNK * 128], f32r, tag=f"w1h{h}")
                    t2w = wpool.tile([128, N_DCHUNK * 128], f32r, tag=f"w2h{h}")
                    nc.sync.dma_start(
                        t1w[:].rearrange("p (d c) -> p d c", d=N_DCHUNK),
                        w1v[:, :, h * 128 : (h + 1) * 128],
                    )
                    nc.sync.dma_start(
                        t2w[:].rearrange("p (d c) -> p d c", d=N_DCHUNK),
                        w2v[:, :, h * 128 : (h + 1) * 128],
                    )
                    w1h.append(t1w)
                    w2h.append(t2w)
                for do in range(N_DCHUNK):
                    tw = wpool.tile([128, N_DCHUNK * 128], f32r, tag=f"woh{do}")
                    nc.sync.dma_start(
                        tw[:].rearrange("p (d c) -> p d c", d=N_DCHUNK),
                        wov[:, :, do * 128 : (do + 1) * 128],
                    )
                    woh.append(tw)

                for bi, (off, NB) in enumerate(blocks):
                    if bi > 0:
                        xsb = load_x(p, off, NB)
                    atsb = atpool.tile([128, N_DCHUNK * NB], f32r, tag="at")
                    for h in range(N_DCHUNK):
                        pa = psA.tile([128, NB], f32, tag="pa")
                        pb = psB.tile([128, NB], f32, tag="pb")
                        for d in range(N_DCHUNK):
                            nc.tensor.matmul(
                                pa[:],
                                w1h[h][:, d * 128 : (d + 1) * 128],
                                xsb[:, d * NB : (d + 1) * NB],
                                start=(d == 0),
                                stop=(d == N_DCHUNK - 1),
                            )
                        for d in range(N_DCHUNK):
                            nc.tensor.matmul(
                                pb[:],
                                w2h[h][:, d * 128 : (d + 1) * 128],
                                xsb[:, d * NB : (d + 1) * NB],
                                start=(d == 0),
                                stop=(d == N_DCHUNK - 1),
                            )
                        t1 = tpool.tile([128, NB], f32, tag="t1")
                        t2 = tpool.tile([128, NB], f32, tag="t2")
                        # t1 on ACT and t2 on DVE run in parallel, shortening
                        # the chain that gates the out-stage matmuls.
                        nc.scalar.activation(t1[:], pa[:], AF.Silu, bias=bsb[:, h : h + 1])
                        nc.vector.tensor_scalar_add(t2[:], pb[:], bsb[:, 8 + h : 9 + h])
                        nc.vector.tensor_mul(atsb[:, h * NB : (h + 1) * NB], t1[:], t2[:])
                    for do in range(N_DCHUNK):
                        pc = psC.tile([128, NB], f32, tag="pc")
                        for h in range(N_DCHUNK):
                            nc.tensor.matmul(
                                pc[:],
                                woh[do][:, h * 128 : (h + 1) * 128],
                                atsb[:, h * NB : (h + 1) * NB],
                                start=(h == 0),
                                stop=(h == N_DCHUNK - 1),
                            )
                        ot = opool.tile([128, NB], f32, tag="ot")
                        nc.scalar.activation(ot[:], pc[:], AF.Identity, bias=bsb[:, 16 + do : 17 + do])
                        nc.sync.dma_start(
                            p["yt"][do * 128 : (do + 1) * 128, off : off + NB], ot[:]
                        )
    nc.compile()
    return nc


def _get_prog(job_sizes):
    key = tuple(job_sizes)
    if key not in _PROG_CACHE:
        _PROG_CACHE[key] = _build(job_sizes)
    return _PROG_CACHE[key]


# --------------------------------------------------------------------------
# Entry point
# --------------------------------------------------------------------------
def kernel(x, w_router, W1, b1, W2, b2, Wout, bout):
    from concourse.bass_utils import run_bass_kernel_spmd

    x = np.asarray(x, np.float32)
    W1 = np.asarray(W1, np.float32)
    b1 = np.asarray(b1, np.float32)
    W2 = np.asarray(W2, np.float32)
    b2 = np.asarray(b2, np.float32)
    Wout = np.asarray(Wout, np.float32)
    bout = np.asarray(bout, np.float32)

    gates, indices = _route(x, w_router)

    # Token lists per expert (ascending token order) and their gate values.
    onehot = indices == np.arange(E)[:, None, None]  # [E, N, K]
    sel = onehot.any(axis=2)  # [E, N]
    gate_of = (gates[None] * onehot).sum(axis=2)  # [E, N]
    toks = [np.nonzero(sel[e])[0] for e in range(E)]
    counts = [len(t) for t in toks]

    groups, sizes = _plan(counts)
    nc = _get_prog(sizes)

    # Per-core inputs.
    in_maps = []
    for c in range(N_CORES):
        m = {}
        for j, S in enumerate(sizes):
            e = groups[j][c]
            t = toks[e]
            xt = np.zeros((D, S), np.float32)
            if len(t):
                xt[:, : len(t)] = x[t].T
            bias = np.zeros((128, 24), np.float32)
            bias[:, 0:8] = b1[e].reshape(8, 128).T
            bias[:, 8:16] = b2[e].reshape(8, 128).T
            bias[:, 16:24] = bout[e].reshape(8, 128).T
            m[f"xt{j}"] = xt
            m[f"w1_{j}"] = np.ascontiguousarray(W1[e])
            m[f"w2_{j}"] = np.ascontiguousarray(W2[e])
            m[f"wo_{j}"] = np.ascontiguousarray(Wout[e])
            m[f"bias{j}"] = bias
        in_maps.append(m)

    trace = bool(os.environ.get("BASS_MOE_TRACE"))
    kw = {}
    if trace:
        kw = dict(trace=True, tmpdir=os.environ.get("BASS_MOE_TRACE_DIR") or None)
    res = run_bass_kernel_spmd(nc, in_maps, list(range(N_CORES)), **kw)
    LAST["result"] = res
    LAST["sizes"] = sizes

    out = np.zeros((N_TOK, D), np.float32)
    for c in range(N_CORES):
        for j in range(len(sizes)):
            e = groups[j][c]
            t = toks[e]
            if not len(t):
                continue
            yt = res.results[c][f"yt{j}"]
            out[t] += yt[:, : len(t)].T * gate_of[e, t][:, None]
    return out
